# revision 1
# baseline (speedup 1.0000x reference)
"""Trainium2 Bass kernel for causal self-attention with cumulative-phase rotary
embedding (nn_CausalSelfAttention_64338610094602).

Sharding: 8 cores = 4 batches x 2 head-groups (tensor-parallel over heads).
Each core computes, for its (batch, 8-head group):
  omega/phi (replicated per batch), QKV projections, rotation + RMSNorm,
  causal attention (transposed-scores layout, max-free softmax), and a
  partial output projection. Host sums the two head-group partials per batch.

All big GEMMs run in float32r (full PE rate at N>=256, ~13-bit mantissa).
The phase/cumsum/trig path is kept in fp32.
"""
import math

import numpy as np
import ml_dtypes

import concourse.mybir as mybir
import concourse.tile as tile
from concourse import bacc
from concourse.bass_utils import run_bass_kernel_spmd

B, T, C = 4, 2048, 2048
H, D, DH = 16, 128, 64
HG = 8          # heads per core (head-group)
GD = HG * D     # group output dims = 1024
NT = T // 512   # 4 t-blocks of 512
NCT = C // 128  # 16 contraction tiles
EPS = 1e-5
SCL = 1.0 / math.sqrt(D)

dt = mybir.dt
AF = mybir.ActivationFunctionType
ALU = mybir.AluOpType

TWO_PI = 6.283185307179586
INV_2PI = 1.0 / TWO_PI
CW1 = float(np.float32(6.28125))
CW2 = float(np.float32(TWO_PI - 6.28125))
CW3 = float(TWO_PI - CW1 - float(np.float32(TWO_PI - 6.28125)))
MAGIC = 12582912.0  # 1.5 * 2^23: fp32 add/sub rounds to nearest int
HALF_PI = 1.5707963267948966
PI = 3.141592653589793

_CACHE = {}


def _round_f32r(x):
    """Round fp32 array to float32r (13-bit mantissa, round-to-nearest-even)."""
    x = np.ascontiguousarray(x, dtype=np.float32)
    b = x.view(np.uint32).copy()
    low = b & np.uint32(0x3FF)
    bb = b & ~np.uint32(0x3FF)
    rnd = (low > 0x200) | ((low == 0x200) & (((bb >> 10) & 1) == 1))
    return (bb + (rnd.astype(np.uint32) << 10)).view(np.float32)


def _build():
    f32, f32r, bf16 = dt.float32, dt.float32r, dt.bfloat16
    nc = bacc.Bacc(None, target_bir_lowering=False)
    with tile.TileContext(nc) as tc:
        xt_d = nc.dram_tensor("xt", (C, T), f32r, kind="ExternalInput")
        wq_d = nc.dram_tensor("wq", (C, GD), f32r, kind="ExternalInput")
        wk_d = nc.dram_tensor("wk", (C, GD), f32r, kind="ExternalInput")
        wv_d = nc.dram_tensor("wv", (C, GD), f32r, kind="ExternalInput")
        wo_d = nc.dram_tensor("wo", (GD, C), f32r, kind="ExternalInput")
        womg_d = nc.dram_tensor("womg", (128, NCT), f32r, kind="ExternalInput")
        b16_d = nc.dram_tensor("b16", (1, 1), f32, kind="ExternalInput")
        logf_d = nc.dram_tensor("logf", (DH, 1), f32, kind="ExternalInput")
        gq_d = nc.dram_tensor("gq", (128, 1), f32, kind="ExternalInput")
        gk_d = nc.dram_tensor("gk", (128, 1), f32, kind="ExternalInput")
        masks_d = nc.dram_tensor("masks", (128, 4 * 512), bf16, kind="ExternalInput")
        onesA_d = nc.dram_tensor("onesA", (128, 1), f32r, kind="ExternalInput")
        onesB_d = nc.dram_tensor("onesB", (1, 128), f32r, kind="ExternalInput")
        ones64_d = nc.dram_tensor("ones64", (1, DH), f32, kind="ExternalInput")
        oneh31_d = nc.dram_tensor("oneh31", (128, 31), f32r, kind="ExternalInput")
        out_d = nc.dram_tensor("out", (T, C), f32, kind="ExternalOutput")

        with tc.tile_pool(name="dram", bufs=1, space="DRAM") as dramp:
            yspill = dramp.tile([128, HG * T], f32r)  # yT per head at col h*T

            with tc.tile_pool(name="const", bufs=1) as constp:
                womg = constp.tile([128, NCT], f32r)
                nc.sync.dma_start(womg[:], womg_d[:])
                b16t = constp.tile([1, 1], f32)
                nc.sync.dma_start(b16t[:], b16_d[:])
                logf = constp.tile([DH, 1], f32)
                nc.sync.dma_start(logf[:], logf_d[:])
                gqt = constp.tile([128, 1], f32)
                nc.sync.dma_start(gqt[:], gq_d[:])
                gkt = constp.tile([128, 1], f32)
                nc.sync.dma_start(gkt[:], gk_d[:])
                onesA = constp.tile([128, 1], f32r)
                nc.sync.dma_start(onesA[:], onesA_d[:])
                onesB = constp.tile([1, 128], f32r)
                nc.sync.dma_start(onesB[:], onesB_d[:])
                ones64 = constp.tile([1, DH], f32)
                nc.sync.dma_start(ones64[:], ones64_d[:])
                epst = constp.tile([1, 1], f32)
                nc.vector.memset(epst[:], EPS)
                eps16 = constp.tile([16, 1], f32)
                nc.vector.memset(eps16[:], EPS)
                oneh31 = constp.tile([128, 31], f32r)
                nc.sync.dma_start(oneh31[:], oneh31_d[:])
                freq = constp.tile([DH, 1], f32)
                nc.scalar.activation(freq[:], logf[:], AF.Exp)

                _main(nc, tc, xt_d, wq_d, wk_d, wv_d, masks_d, yspill,
                      womg, b16t, gqt, gkt, onesA, onesB, ones64, eps16,
                      oneh31, freq)

                # ---- P3: output projection out = yall^T @ wo ----
                with tc.tile_pool(name="p3", bufs=1) as p3, \
                     tc.tile_pool(name="p3o", bufs=3) as p3o, \
                     tc.tile_pool(name="p3ps", bufs=4, space="PSUM") as p3ps:
                    yall = p3.tile([128, HG * T], f32r)
                    wosb = p3.tile([128, HG * C], f32r)
                    for h in range(HG):
                        nc.sync.dma_start(yall[:, h * T:(h + 1) * T],
                                          yspill[:, h * T:(h + 1) * T])
                        nc.sync.dma_start(wosb[:, h * C:(h + 1) * C],
                                          wo_d[h * 128:(h + 1) * 128, :])
                    for ti in range(T // 128):
                        for cb in range(C // 512):
                            ops = p3ps.tile([128, 512], f32, tag="o")
                            for h in range(HG):
                                nc.tensor.matmul(
                                    ops[:],
                                    yall[:, h * T + ti * 128:h * T + (ti + 1) * 128],
                                    wosb[:, h * C + cb * 512:h * C + (cb + 1) * 512],
                                    start=(h == 0), stop=(h == HG - 1))
                            osb = p3o.tile([128, 512], f32, tag="osb")
                            nc.vector.tensor_copy(osb[:], ops[:])
                            nc.sync.dma_start(
                                out_d[ti * 128:(ti + 1) * 128, cb * 512:(cb + 1) * 512],
                                osb[:])
    nc.compile()
    return nc


def _main(nc, tc, xt_d, wq_d, wk_d, wv_d, masks_d, yspill,
          womg, b16t, gqt, gkt, onesA, onesB, ones64, eps16, oneh31, freq):
    f32, f32r, bf16 = dt.float32, dt.float32r, dt.bfloat16

    with tc.tile_pool(name="big", bufs=1) as bigp, \
         tc.tile_pool(name="xtp", bufs=1) as xtp:
        trig = bigp.tile([128, T], f32)       # [0:64]=cos, [64:128]=sin
        masks = bigp.tile([128, 4 * 512], bf16)
        nc.sync.dma_start(masks[:], masks_d[:])

        xts = xtp.tile([128, NCT * T], f32r)  # c-tile i at cols [i*T, (i+1)*T)
        for i in range(NCT):
            nc.sync.dma_start(xts[:, i * T:(i + 1) * T],
                              xt_d[i * 128:(i + 1) * 128, :])

        # ---- P1: omega -> phi -> trig ----
        with tc.tile_pool(name="p1", bufs=1) as p1, \
             tc.tile_pool(name="p1b", bufs=2) as p1b, \
             tc.tile_pool(name="p1ps", bufs=2, space="PSUM") as p1ps:
            omega = p1.tile([1, T], f32)
            for J in range(NT):
                omps = p1ps.tile([1, 512], f32, tag="om")
                for i in range(NCT):
                    nc.tensor.matmul(
                        omps[:], womg[:, i:i + 1],
                        xts[:, i * T + J * 512:i * T + J * 512 + 512],
                        start=(i == 0), stop=(i == NCT - 1))
                nc.scalar.activation(omega[:, J * 512:(J + 1) * 512], omps[:],
                                     AF.Sigmoid, scale=1.0 / 16.0, bias=b16t[:])
            incl = p1.tile([1, T], f32)
            nc.vector.tensor_tensor_scan(incl[:], omega[:], omega[:], 0.0,
                                         ALU.add, ALU.bypass)
            phi = p1.tile([1, T], f32)
            nc.vector.tensor_sub(phi[:], incl[:], omega[:])
            for J in range(NT):
                sl = slice(J * 512, (J + 1) * 512)
                phps = p1ps.tile([DH, 512], f32, tag="phib")
                nc.tensor.matmul(phps[:], ones64[:], phi[:, sl],
                                 start=True, stop=True)
                ang = p1b.tile([DH, 512], f32, tag="ang")
                nc.vector.tensor_scalar(ang[:], phps[:], freq[:], None, op0=ALU.mult)
                mm = p1b.tile([DH, 512], f32, tag="mm")
                nc.vector.tensor_scalar(mm[:], ang[:], INV_2PI, MAGIC,
                                        op0=ALU.mult, op1=ALU.add)
                kk = p1b.tile([DH, 512], f32, tag="kk")
                nc.vector.tensor_scalar_add(kk[:], mm[:], -MAGIC)
                red = p1b.tile([DH, 512], f32, tag="red")
                nc.vector.cody_waite_cascade(red[:], ang[:], kk[:], CW1, CW2, CW3)
                red2 = p1b.tile([DH, 512], f32, tag="red2")
                nc.vector.add_range_wrap(red2[:], red[:], HALF_PI, PI, TWO_PI)
                nc.scalar.activation(trig[0:DH, sl], red2[:], AF.Sin)   # cos
                nc.scalar.activation(trig[DH:128, sl], red[:], AF.Sin)  # sin

        # ---- P2: per head-pair: QKV + rot/norm + attention ----
        with tc.tile_pool(name="qkv", bufs=1) as qkvp, \
             tc.tile_pool(name="wst", bufs=3) as wst, \
             tc.tile_pool(name="sc512", bufs=1) as sc512, \
             tc.tile_pool(name="rows", bufs=1) as rowsp:
            for pair in range(4):
                q_sb = qkvp.tile([128, 2 * T], f32r, tag="q", name=f"q_{pair}")
                k_sb = qkvp.tile([128, 2 * T], f32r, tag="k", name=f"k_{pair}")
                v_sb = qkvp.tile([128, 16 * 256], f32r, tag="v", name=f"v_{pair}")

                # --- 2a: q/k for both heads (4 J banks); ssq comes straight
                # from the pre-rotation tile (rotation is norm-preserving);
                # gamma rides the ACT eviction copy; rstd batched per pair. ---
                with tc.tile_pool(name=f"psA_{pair}", bufs=1, space="PSUM") as psA, \
                     tc.tile_pool(name=f"psS_{pair}", bufs=1, space="PSUM") as psS, \
                     tc.tile_pool(name=f"psR_{pair}", bufs=2, space="PSUM") as psR:
                    ssqps = psS.tile([16, 512], f32, tag="ssq",
                                     name=f"ssqps_{pair}")
                    site = 0
                    for wi, (w_d, gam, dest) in enumerate(
                            ((wq_d, gqt, q_sb), (wk_d, gkt, k_sb))):
                        for hl in range(2):
                            h = pair * 2 + hl
                            qps = {}
                            for J in range(NT):
                                qp = psA.tile([128, 512], f32, tag=f"q{J}",
                                              name=f"qp_{pair}_{wi}_{hl}_{J}")
                                qps[J] = qp
                            for i in range(NCT):
                                wt = wst.tile([128, 128], f32r, tag="w")
                                nc.sync.dma_start(
                                    wt[:],
                                    w_d[i * 128:(i + 1) * 128,
                                        h * 128:(h + 1) * 128])
                                for J in range(NT):
                                    nc.tensor.matmul(
                                        qps[J][:], wt[:],
                                        xts[:, i * T + J * 512:i * T + J * 512 + 512],
                                        start=(i == 0), stop=(i == NCT - 1))
                            for J in range(NT):
                                rot = _rotate(nc, sc512, qps[J], trig, J)
                                sq = sc512.tile([128, 512], f32r, tag="ta",
                                                name=f"sq_{pair}_{site}")
                                nc.scalar.activation(sq[:], qps[J][:], AF.Square)
                                nc.tensor.matmul(
                                    ssqps[:], oneh31[:, 15 - site:31 - site],
                                    sq[:],
                                    start=(site == 0), stop=(site == 15))
                                dcol = hl * T + J * 512
                                nc.scalar.activation(
                                    dest[:, dcol:dcol + 512], rot[:], AF.Copy,
                                    scale=gam[:])
                                site += 1
                    # batched rstd = exp(-0.5 * ln(ssq/128 + eps)) for 16 sites
                    lnt = sc512.tile([16, 512], f32, tag="ta",
                                     name=f"lnt_{pair}")
                    nc.scalar.activation(lnt[:], ssqps[:], AF.Ln,
                                         scale=1.0 / 128.0, bias=eps16[:])
                    rstd = sc512.tile([16, 512], f32r, tag="tb",
                                      name=f"rstd_{pair}")
                    nc.scalar.activation(rstd[:], lnt[:], AF.Exp, scale=-0.5)
                # --- 2b: v for both heads (N=256 wide) ---
                with tc.tile_pool(name=f"ps2b_{pair}", bufs=1, space="PSUM") as psb:
                    for half in range(2):
                        vps = []
                        for t in range(8):
                            vp = psb.tile([128, 256], f32, tag=f"v{t}",
                                          name=f"vp_{pair}_{half}_{t}")
                            vps.append(vp)
                        for i in range(NCT):
                            wvt = wst.tile([128, 256], f32r, tag="wv")
                            nc.sync.dma_start(
                                wvt[:],
                                wv_d[i * 128:(i + 1) * 128,
                                     pair * 256:(pair + 1) * 256])
                            for t in range(8):
                                tt = half * 8 + t
                                nc.tensor.matmul(
                                    vps[t][:],
                                    xts[:, i * T + tt * 128:i * T + (tt + 1) * 128],
                                    wvt[:],
                                    start=(i == 0), stop=(i == NCT - 1))
                        for t in range(8):
                            tt = half * 8 + t
                            nc.vector.tensor_copy(
                                v_sb[:, tt * 256:(tt + 1) * 256], vps[t][:])

                # --- deferred q/k normalize (rstd broadcast): runs after
                # the v matmuls so the PE never waits on the Ln/Exp chain ---
                with tc.tile_pool(name=f"psN_{pair}", bufs=2, space="PSUM") as psR:
                    site = 0
                    for wi in range(2):
                        dest = (q_sb, k_sb)[wi]
                        for hl in range(2):
                            for J in range(NT):
                                rrow = rowsp.tile([1, 512], f32r,
                                                  tag="r1" if site % 2 == 0 else "r0",
                                                  name=f"rrow_{pair}_{site}")
                                nc.sync.dma_start(rrow[:], rstd[site:site + 1, :])
                                rbps = psR.tile([128, 512], f32, tag="rb",
                                                name=f"rb2a_{pair}_{site}")
                                nc.tensor.matmul(rbps[:], onesB[:], rrow[:],
                                                 start=True, stop=True)
                                dcol = hl * T + J * 512
                                nc.vector.tensor_tensor(
                                    dest[:, dcol:dcol + 512],
                                    dest[:, dcol:dcol + 512], rbps[:],
                                    op=ALU.mult)
                                site += 1


                # --- 2c: attention per head; each J's softmax epilogue is
                # emitted inside the next J's matmul stream so the in-order
                # PE queue never stalls on the recip -> broadcast chain ---
                with tc.tile_pool(name=f"ps2c_{pair}", bufs=2, space="PSUM") as psc:
                    pend = [None]

                    def epilogue(yps, dps, hl, J):
                        h = pair * 2 + hl
                        rcf = rowsp.tile([1, 512], f32, tag="r0",
                                         name=f"rcf_{pair}_{hl}_{J}")
                        nc.vector.reciprocal_approx_fast(out=rcf[:], in_=dps[:])
                        recip = rowsp.tile([1, 512], f32r, tag="r1",
                                           name=f"recip_{pair}_{hl}_{J}")
                        nc.vector.tensor_copy(recip[:], rcf[:])
                        rbps = psc.tile([128, 512], f32, tag="rb", bufs=1,
                                        name=f"rbps_{pair}_{hl}_{J}")
                        nc.tensor.matmul(rbps[:], onesB[:], recip[:],
                                         start=True, stop=True)
                        rbsb = sc512.tile([128, 512], f32, tag="tb",
                                          name=f"rbsb_{pair}_{hl}_{J}")
                        nc.scalar.copy(rbsb[:], rbps[:])
                        yt = sc512.tile([128, 512], f32r, tag="ta",
                                        name=f"yt_{pair}_{hl}_{J}")
                        nc.vector.tensor_tensor(yt[:], yps[:], rbsb[:],
                                                op=ALU.mult)
                        nc.sync.dma_start(
                            yspill[:, h * T + J * 512:h * T + (J + 1) * 512],
                            yt[:])

                    for hl in range(2):
                        for J in range(NT):
                            nI = 4 * J + 4
                            yps = psc.tile([128, 512], f32, tag="y",
                                           name=f"yps_{pair}_{hl}_{J}")
                            dps = psc.tile([1, 512], f32, tag="den",
                                           name=f"dps_{pair}_{hl}_{J}")
                            for I in range(nI):
                                sps = psc.tile([128, 512], f32, tag="s", bufs=3,
                                               name=f"sps_{pair}_{hl}_{J}_{I}")
                                nc.tensor.matmul(
                                    sps[:],
                                    k_sb[:, hl * T + I * 128:hl * T + (I + 1) * 128],
                                    q_sb[:, hl * T + J * 512:hl * T + (J + 1) * 512],
                                    start=True, stop=True)
                                ex = sc512.tile([128, 512], f32r,
                                                tag="ex" if I % 2 == 0 else "ex2",
                                                name=f"ex_{pair}_{hl}_{J}_{I}")
                                nc.scalar.activation(ex[:], sps[:], AF.Exp, scale=SCL)
                                if I >= 4 * J:
                                    r = I - 4 * J
                                    exm = sc512.tile([128, 512], f32r, tag="rot",
                                                     name=f"exm_{pair}_{hl}_{J}_{I}")
                                    nc.vector.tensor_tensor(
                                        exm[:], ex[:], masks[:, r * 512:(r + 1) * 512],
                                        op=ALU.mult)
                                    use = exm
                                else:
                                    use = ex
                                nc.tensor.matmul(
                                    yps[:],
                                    v_sb[:, I * 256 + hl * 128:I * 256 + hl * 128 + 128],
                                    use[:], start=(I == 0), stop=(I == nI - 1))
                                nc.tensor.matmul(
                                    dps[:], onesA[:], use[:],
                                    start=(I == 0), stop=(I == nI - 1))
                                if I == 1 and pend[0] is not None:
                                    pend[0]()
                                    pend[0] = None
                            pend[0] = (lambda yps=yps, dps=dps, hl=hl, J=J:
                                       epilogue(yps, dps, hl, J))
                    pend[0]()
                    pend[0] = None


def _rotate(nc, sc512, qps, trig, J):
    """Rotate (cumulative-phase RoPE) one (128, 512) projection PSUM tile.

    trig[0:64]=cos, [64:128]=sin for this J. Returns the rotated f32 tile.
    Ordered so the PSUM bank is released after the first 3 DVE ops."""
    f32 = dt.float32
    sl = slice(J * 512, (J + 1) * 512)
    ta = sc512.tile([DH, 512], f32, tag="ta")      # q1*cos
    tb = sc512.tile([DH, 512], f32, tag="tb")      # q2*sin
    tcc = sc512.tile([DH, 512], f32, tag="ex")     # q2*cos (ex slot: 2c-only)
    td = sc512.tile([DH, 512], f32, tag="ex2")     # q1*sin (ex2 slot: 2c-only)
    rot = sc512.tile([128, 512], f32, tag="rot")
    nc.vector.tensor_tensor(ta[:], qps[0:DH, :], trig[0:DH, sl], op=ALU.mult)
    nc.vector.tensor_tensor(tb[:], qps[DH:128, :], trig[DH:128, sl], op=ALU.mult)
    nc.vector.tensor_tensor(tcc[:], qps[DH:128, :], trig[0:DH, sl], op=ALU.mult)
    nc.vector.tensor_tensor(td[:], qps[0:DH, :], trig[DH:128, sl], op=ALU.mult)
    # PSUM bank free from here on
    nc.vector.tensor_add(rot[0:DH, :], ta[:], tb[:])
    nc.vector.tensor_sub(rot[DH:128, :], tcc[:], td[:])
    return rot


def _host_prep(inputs):
    x = np.asarray(inputs["x"], dtype=np.float32)
    Wq = np.asarray(inputs["Wq"], dtype=np.float32)
    Wk = np.asarray(inputs["Wk"], dtype=np.float32)
    Wv = np.asarray(inputs["Wv"], dtype=np.float32)
    Wo = np.asarray(inputs["Wo"], dtype=np.float32)
    w_omega = np.asarray(inputs["w_omega"], dtype=np.float32)
    b_omega = np.asarray(inputs["b_omega"], dtype=np.float32)
    log_freq = np.asarray(inputs["log_freq"], dtype=np.float32)
    q_gamma = np.asarray(inputs["q_gamma"], dtype=np.float32)
    k_gamma = np.asarray(inputs["k_gamma"], dtype=np.float32)

    womg = _round_f32r(w_omega.reshape(NCT, 128).T)  # [p, i] = w_omega[i*128+p]
    b16 = (b_omega / 16.0).reshape(1, 1).astype(np.float32)
    logf = log_freq.reshape(DH, 1)
    gq = q_gamma.reshape(128, 1)
    gk = k_gamma.reshape(128, 1)
    p = np.arange(128)[:, None]
    c = np.arange(512)[None, :]
    masks = np.concatenate(
        [((p + r * 128) <= c).astype(np.float32) for r in range(4)], axis=1
    ).astype(ml_dtypes.bfloat16)
    onesA = np.ones((128, 1), dtype=np.float32)
    onesB = np.ones((1, 128), dtype=np.float32)
    ones64 = np.ones((1, DH), dtype=np.float32)
    oneh31 = np.zeros((128, 31), dtype=np.float32)
    oneh31[:, 15] = 1.0

    in_maps = []
    for core in range(8):
        b, g = core // 2, core % 2
        in_maps.append({
            "xt": _round_f32r(x[b].T),
            "wq": _round_f32r(Wq[g * GD:(g + 1) * GD, :].T),
            "wk": _round_f32r(Wk[g * GD:(g + 1) * GD, :].T),
            "wv": _round_f32r(Wv[g * GD:(g + 1) * GD, :].T),
            "wo": _round_f32r(Wo[:, g * GD:(g + 1) * GD].T),
            "womg": womg, "b16": b16, "logf": logf, "gq": gq, "gk": gk,
            "masks": masks, "onesA": onesA, "onesB": onesB, "ones64": ones64,
            "oneh31": oneh31,
        })
    return in_maps


def kernel(**inputs) -> np.ndarray:
    if "nc" not in _CACHE:
        _CACHE["nc"] = _build()
    nc = _CACHE["nc"]
    in_maps = _host_prep(inputs)
    res = run_bass_kernel_spmd(nc, in_maps, core_ids=list(range(8)))
    out = np.empty((B, T, C), dtype=np.float32)
    for b in range(B):
        out[b] = res.results[2 * b]["out"] + res.results[2 * b + 1]["out"]
    return out



# revision 30
# speedup vs baseline: 1.3183x; 1.3183x over previous
"""Trainium2 Bass kernel for causal self-attention with cumulative-phase rotary
embedding (nn_CausalSelfAttention_64338610094602).

Sharding: 8 cores = 4 batches x 2 head-groups (tensor-parallel over heads).
Each core computes, for its (batch, 8-head group):
  omega/phi (replicated per batch), QKV projections, rotation + RMSNorm,
  causal attention (transposed-scores layout, max-free softmax), and a
  partial output projection. Host sums the two head-group partials per batch.

v2 design notes:
  - All big GEMM operands in bf16 (matmul rate keys off the moving operand;
    bf16 is 1 cycle/row at any N). PSUM accumulation stays fp32.
  - Rotation: gamma folded into per-J trig tiles; 4 DVE ops per site-J
    (A = qps*trigA, swapped-half B twice, one add writes q_sb/k_sb directly
    in bf16). No separate ACT eviction.
  - RMSNorm: Square on ACT, partition_all_reduce on GpSimd (PE no longer
    computes sum-of-squares); batched Ln/Exp per site on the replicated
    [128, 2048] reduce output produces the broadcast rstd directly.
  - Causal mask folded into the PE accumulation: for diagonal score tiles,
    one extra matmul (tril stationary x -1e9 step moving) adds -1e9*count
    to masked entries, so exp() gives exact zeros. No DVE mask multiply.
  - 2c softmax: scores for two key tiles accumulate into one [128,1024]
    PSUM tile (2 banks); a single wide ACT Exp halves ACT per-op overhead.
    Software-pipelined: next pair's scores are issued before this pair's
    yps/dps consumers so the in-order PE queue never waits on the Exp.
  - Softmax denominator reciprocal broadcast via GpSimd partition_broadcast
    (no K=1 PE matmuls); y stays resident in SBUF in bf16 (no DRAM spill).
  - PSUM budget: tagS 2x[128,1024] (4 banks) shared by projections/v/scores,
    tagY 2x[128,512] (yps + P3), tagD 2x[1,512] (omega + denominators).
"""
import math

import numpy as np
import ml_dtypes

import concourse.mybir as mybir
import concourse.tile as tile
from concourse import bacc
from concourse.bass_utils import run_bass_kernel_spmd

B, T, C = 4, 2048, 2048
H, D, DH = 16, 128, 64
HG = 8          # heads per core (head-group)
GD = HG * D     # group output dims = 1024
NT = T // 512   # 4 query blocks of 512
NCT = C // 128  # 16 contraction tiles
EPS = 1e-5
SCL = 1.0 / math.sqrt(D)
NEG = -1.0e9

dt = mybir.dt
AF = mybir.ActivationFunctionType
ALU = mybir.AluOpType

TWO_PI = 6.283185307179586
INV_2PI = 1.0 / TWO_PI
CW1 = float(np.float32(6.28125))
CW2 = float(np.float32(TWO_PI - 6.28125))
CW3 = float(TWO_PI - CW1 - float(np.float32(TWO_PI - 6.28125)))
MAGIC = 12582912.0  # 1.5 * 2^23: fp32 add/sub rounds to nearest int
HALF_PI = 1.5707963267948966
PI = 3.141592653589793

_CACHE = {}
DEBUG = False


def _build():
    from concourse import bass_isa
    red_add = bass_isa.ReduceOp.add

    f32, bf16 = dt.float32, dt.bfloat16
    nc = bacc.Bacc(None, target_bir_lowering=False)
    with tile.TileContext(nc) as tc:
        xt_d = nc.dram_tensor("xt", (C, T), bf16, kind="ExternalInput")
        wq_d = nc.dram_tensor("wq", (C, GD), bf16, kind="ExternalInput")
        wk_d = nc.dram_tensor("wk", (C, GD), bf16, kind="ExternalInput")
        wv_d = nc.dram_tensor("wv", (C, GD), bf16, kind="ExternalInput")
        wo_d = nc.dram_tensor("wo", (GD, C), bf16, kind="ExternalInput")
        womg_d = nc.dram_tensor("womg", (128, NCT), bf16, kind="ExternalInput")
        b16_d = nc.dram_tensor("b16", (1, 1), f32, kind="ExternalInput")
        logf2_d = nc.dram_tensor("logf2", (128, 1), f32, kind="ExternalInput")
        gq_d = nc.dram_tensor("gq", (128, 1), f32, kind="ExternalInput")
        gqB_d = nc.dram_tensor("gqB", (128, 1), f32, kind="ExternalInput")
        gk_d = nc.dram_tensor("gk", (128, 1), f32, kind="ExternalInput")
        gkB_d = nc.dram_tensor("gkB", (128, 1), f32, kind="ExternalInput")
        trilA_d = nc.dram_tensor("trilA", (128, 128), bf16, kind="ExternalInput")
        maskB_d = nc.dram_tensor("maskB", (128, 4 * 512), bf16, kind="ExternalInput")
        onesA_d = nc.dram_tensor("onesA", (128, 1), bf16, kind="ExternalInput")
        out_d = nc.dram_tensor("out", (T, C), f32, kind="ExternalOutput")
        dbg = {}
        if DEBUG:
            dbg["trig"] = nc.dram_tensor("dbg_trig", (128, 4 * T), bf16,
                                         kind="ExternalOutput")
            dbg["q"] = nc.dram_tensor("dbg_q", (128, 2 * T), bf16,
                                      kind="ExternalOutput")
            dbg["k"] = nc.dram_tensor("dbg_k", (128, 2 * T), bf16,
                                      kind="ExternalOutput")
            dbg["v"] = nc.dram_tensor("dbg_v", (128, 16 * 256), bf16,
                                      kind="ExternalOutput")
            dbg["y"] = nc.dram_tensor("dbg_y", (128, HG * T), bf16,
                                      kind="ExternalOutput")
            dbg["den"] = nc.dram_tensor("dbg_den", (1, 8 * 512), f32,
                                        kind="ExternalOutput")
            dbg["rnb"] = nc.dram_tensor("dbg_rnb", (128, 4 * T), bf16,
                                        kind="ExternalOutput")

        with tc.tile_pool(name="const", bufs=1) as constp, \
             tc.tile_pool(name="dram", bufs=1, space="DRAM") as dramp, \
             tc.tile_pool(name="psp", bufs=1, space="PSUM") as psp:

            # ---- constants ----
            womg = constp.tile([128, NCT], bf16)
            nc.sync.dma_start(womg[:], womg_d[:])
            b16t = constp.tile([1, 1], f32)
            nc.sync.dma_start(b16t[:], b16_d[:])
            logf2 = constp.tile([128, 1], f32)
            nc.sync.dma_start(logf2[:], logf2_d[:])
            gq = constp.tile([128, 1], f32)
            nc.sync.dma_start(gq[:], gq_d[:])
            gqB = constp.tile([128, 1], f32)
            nc.sync.dma_start(gqB[:], gqB_d[:])
            gk = constp.tile([128, 1], f32)
            nc.sync.dma_start(gk[:], gk_d[:])
            gkB = constp.tile([128, 1], f32)
            nc.sync.dma_start(gkB[:], gkB_d[:])
            trilA = constp.tile([128, 128], bf16)
            nc.sync.dma_start(trilA[:], trilA_d[:])
            maskB = constp.tile([128, 4 * 512], bf16)
            nc.sync.dma_start(maskB[:], maskB_d[:])
            onesA = constp.tile([128, 1], bf16)
            nc.sync.dma_start(onesA[:], onesA_d[:])
            freq2 = constp.tile([128, 1], f32)
            nc.scalar.activation(freq2[:], logf2[:], AF.Exp)
            eps128 = constp.tile([128, 1], f32)
            nc.vector.memset(eps128[:], EPS)

            y_d = dramp.tile([128, HG * T], bf16)  # yT per head at col h*T

            with tc.tile_pool(name="big", bufs=1) as bigp, \
                 tc.tile_pool(name="xtp", bufs=1) as xtp, \
                 tc.tile_pool(name="wstp", bufs=1) as wstp, \
                 tc.tile_pool(name="scp", bufs=1) as scp, \
                 tc.tile_pool(name="rowp", bufs=1) as rowp:
                # pair-persistent SBUF state
                q_sb = bigp.tile([128, 2 * T], bf16)   # [D, hl*T + t]
                k_sb = bigp.tile([128, 2 * T], bf16)
                v_sb = bigp.tile([128, 16 * 256], bf16)  # key tile tt at tt*256
                trigA_q = bigp.tile([128, T], bf16)
                trigB_q = bigp.tile([128, T], bf16)
                trigA_k = bigp.tile([128, T], bf16)
                trigB_k = bigp.tile([128, T], bf16)
                _main(nc, tc, xt_d, wq_d, wk_d, wv_d, red_add,
                      xtp, wstp, scp, rowp, psp,
                      womg, b16t, freq2, gq, gqB, gk, gkB, trilA, maskB,
                      onesA, eps128, q_sb, k_sb, v_sb, y_d,
                      trigA_q, trigB_q, trigA_k, trigB_k, dbg)

            # ---- P3: output projection out = y^T W_o (partial over heads) ----
            with tc.tile_pool(name="p3w", bufs=1) as p3w, \
                 tc.tile_pool(name="p3o", bufs=1) as p3o:
                wosb = p3w.tile([128, HG * C], bf16, name="wosb")
                for cb in range(C // 512):
                    for hh in range(HG):
                        nc.sync.dma_start(
                            wosb[:, hh * C + cb * 512:hh * C + (cb + 1) * 512],
                            wo_d[hh * 128:(hh + 1) * 128,
                                 cb * 512:(cb + 1) * 512])
                y_slots = [None, None]

                def issue_y(ti):
                    yti = p3w.tile([128, HG * 128], bf16, tag="yt", bufs=2,
                                   name=f"yti_{ti}")
                    for hh in range(HG):
                        nc.sync.dma_start(
                            yti[:, hh * 128:(hh + 1) * 128],
                            y_d[:, hh * T + ti * 128:hh * T + (ti + 1) * 128])
                    y_slots[ti % 2] = yti

                issue_y(0)
                for ti in range(T // 128):
                    if ti + 1 < T // 128:
                        issue_y(ti + 1)
                    yti = y_slots[ti % 2]
                    for cb in range(C // 512):
                        ops = psp.tile([128, 512], f32, tag="y", bufs=2,
                                       name=f"ops_{cb}_{ti}")
                        for hh in range(HG):
                            nc.tensor.matmul(
                                ops[:],
                                yti[:, hh * 128:(hh + 1) * 128],
                                wosb[:, hh * C + cb * 512:hh * C + (cb + 1) * 512],
                                start=(hh == 0), stop=(hh == HG - 1))
                        osb = p3o.tile([128, 512], f32, tag="osb", bufs=3)
                        nc.scalar.copy(osb[:], ops[:])
                        nc.sync.dma_start(
                            out_d[ti * 128:(ti + 1) * 128,
                                  cb * 512:(cb + 1) * 512],
                            osb[:])
    nc.compile()
    return nc


def _main(nc, tc, xt_d, wq_d, wk_d, wv_d, red_add,
          xtp, wstp, scp, rowp, psp,
          womg, b16t, freq2, gq, gqB, gk, gkB, trilA, maskB,
          onesA, eps128, q_sb, k_sb, v_sb, y_d,
          trigA_q, trigB_q, trigA_k, trigB_k, dbg):
    f32, bf16 = dt.float32, dt.bfloat16

    xts = xtp.tile([128, NCT * T], bf16)  # c-tile i at cols [i*T,(i+1)*T)
    for i in range(NCT):
        nc.sync.dma_start(xts[:, i * T:(i + 1) * T],
                          xt_d[i * 128:(i + 1) * 128, :])

    # ---- P1: omega -> phi -> trig (PE does only the omega matvecs) ----
    with tc.tile_pool(name="p1p", bufs=1) as p1p:
        omega = rowp.tile([1, T], f32, tag="om")
        for J in range(NT):
            omps = psp.tile([1, 512], f32, tag="d", bufs=2, name=f"omps_{J}")
            for i in range(NCT):
                nc.tensor.matmul(
                    omps[:], womg[:, i:i + 1],
                    xts[:, i * T + J * 512:i * T + J * 512 + 512],
                    start=(i == 0), stop=(i == NCT - 1))
            nc.scalar.activation(omega[:, J * 512:(J + 1) * 512], omps[:],
                                 AF.Sigmoid, scale=1.0 / 16.0, bias=b16t[:])
        incl = rowp.tile([1, T], f32, tag="incl")
        nc.vector.tensor_tensor_scan(incl[:], omega[:], omega[:], 0.0,
                                     ALU.add, ALU.bypass)
        phi = rowp.tile([1, T], f32, tag="phi")
        nc.vector.tensor_sub(phi[:], incl[:], omega[:])
        for J in range(NT):
            sl = slice(J * 512, (J + 1) * 512)
            phi2 = p1p.tile([128, 512], f32, tag="p1", bufs=4,
                            name=f"phi2_{J}")
            nc.gpsimd.partition_broadcast(phi2[:], phi[:, sl])
            ang = p1p.tile([128, 512], f32, tag="p1", bufs=4, name=f"ang_{J}")
            nc.vector.tensor_scalar(ang[:], phi2[:], freq2[:], None,
                                    op0=ALU.mult)
            mm = p1p.tile([128, 512], f32, tag="p1", bufs=4, name=f"mm_{J}")
            nc.vector.tensor_scalar(mm[:], ang[:], INV_2PI, MAGIC,
                                    op0=ALU.mult, op1=ALU.add)
            kk = p1p.tile([128, 512], f32, tag="p1", bufs=4, name=f"kk_{J}")
            nc.vector.tensor_scalar_add(kk[:], mm[:], -MAGIC)
            red = p1p.tile([128, 512], f32, tag="p1", bufs=4, name=f"red_{J}")
            nc.vector.cody_waite_cascade(red[:], ang[:], kk[:], CW1, CW2, CW3)
            red2 = p1p.tile([128, 512], f32, tag="p1", bufs=4,
                            name=f"red2_{J}")
            nc.vector.add_range_wrap(red2[:], red[:], HALF_PI, PI, TWO_PI)
            sinr = p1p.tile([128, 512], f32, tag="p1", bufs=4,
                            name=f"sinr_{J}")
            nc.scalar.activation(sinr[:], red[:], AF.Sin)
            cosr = p1p.tile([128, 512], f32, tag="p1", bufs=4,
                            name=f"cosr_{J}")
            nc.scalar.activation(cosr[:], red2[:], AF.Sin)
            # gamma-folded trig tiles (bf16)
            nc.scalar.activation(trigA_q[:, sl], cosr[:], AF.Copy, scale=gq[:])
            nc.scalar.activation(trigB_q[:, sl], sinr[:], AF.Copy, scale=gqB[:])
            nc.scalar.activation(trigA_k[:, sl], cosr[:], AF.Copy, scale=gk[:])
            nc.scalar.activation(trigB_k[:, sl], sinr[:], AF.Copy, scale=gkB[:])
        if dbg:
            for ii, tt_ in enumerate((trigA_q, trigB_q, trigA_k, trigB_k)):
                nc.sync.dma_start(dbg["trig"][:, ii * T:(ii + 1) * T], tt_[:])

    # ---- P2 per pair ----
    pend_norm = [None]
    pend_epi = [None]

    def flush(pend):
        if pend[0] is not None:
            pend[0]()
            pend[0] = None

    sites = [(pair, wi, hl) for pair in range(4) for wi in range(2)
             for hl in range(2)]
    wp_slots = [None, None]

    def issue_panel(si):
        pair, wi, hl = sites[si]
        h = pair * 2 + hl
        w_d = (wq_d, wk_d)[wi]
        wp = wstp.tile([128, NCT * 128], bf16, tag="wp", bufs=2,
                       name=f"wp_{si}")
        for i in range(NCT):
            nc.sync.dma_start(
                wp[:, i * 128:(i + 1) * 128],
                w_d[i * 128:(i + 1) * 128, h * 128:(h + 1) * 128])
        wp_slots[si % 2] = wp

    issue_panel(0)

    for pair in range(4):
        # wv panel for this pair (resident; streamed during 2a)
        wvp = wstp.tile([128, NCT * 256], bf16, tag="wvp", bufs=1,
                        name=f"wvp_{pair}")
        for i in range(NCT):
            nc.sync.dma_start(
                wvp[:, i * 256:(i + 1) * 256],
                wv_d[i * 128:(i + 1) * 128, pair * 256:(pair + 1) * 256])

        # --- 2a: q/k for both heads ---
        for wi in range(2):
            for hl in range(2):
                si = pair * 4 + wi * 2 + hl
                if si + 1 < len(sites):
                    issue_panel(si + 1)
                wp = wp_slots[si % 2]
                trigA = (trigA_q, trigA_k)[wi]
                trigB = (trigB_q, trigB_k)[wi]
                dest = (q_sb, k_sb)[wi]
                sqs = []
                for Jp in range(2):
                    qps2 = psp.tile([128, 1024], f32, tag="s", bufs=2,
                                    name=f"qps2_{si}_{Jp}")
                    for i in range(NCT):
                        for Jh in range(2):
                            J = 2 * Jp + Jh
                            nc.tensor.matmul(
                                qps2[:, Jh * 512:(Jh + 1) * 512],
                                wp[:, i * 128:(i + 1) * 128],
                                xts[:, i * T + J * 512:i * T + J * 512 + 512],
                                start=(i == 0), stop=(i == NCT - 1))
                    for Jh in range(2):
                        J = 2 * Jp + Jh
                        qsl = qps2[:, Jh * 512:(Jh + 1) * 512]
                        sl = slice(J * 512, (J + 1) * 512)
                        dcol = hl * T + J * 512
                        # rotation: A + swapped-half B, gamma folded in trig
                        A = scp.tile([128, 512], f32, tag="ra", bufs=2,
                                     name=f"A_{si}_{J}")
                        nc.vector.tensor_tensor(A[:], qsl, trigA[:, sl],
                                                op=ALU.mult)
                        Bt = scp.tile([128, 512], f32, tag="rb", bufs=2,
                                      name=f"Bt_{si}_{J}")
                        nc.vector.tensor_tensor(
                            Bt[0:DH, :],
                            qps2[DH:128, Jh * 512:(Jh + 1) * 512],
                            trigB[0:DH, sl], op=ALU.mult)
                        nc.vector.tensor_tensor(
                            Bt[DH:128, :],
                            qps2[0:DH, Jh * 512:(Jh + 1) * 512],
                            trigB[DH:128, sl], op=ALU.mult)
                        nc.vector.tensor_add(
                            dest[:, dcol:dcol + 512], A[:], Bt[:])
                        # sum-of-squares path (rotation preserves norms)
                        sq = scp.tile([128, 512], bf16, tag="sq", bufs=4,
                                      name=f"sq_{si}_{J}")
                        nc.scalar.activation(sq[:], qsl, AF.Square)
                        sqs.append((J, sq))
                # GpSimd partition reduce -> replicated ssq -> rstd broadcast
                rnbs = []
                for J, sq in sqs:
                    srr = scp.tile([128, 512], f32, tag="sr", bufs=2,
                                   name=f"sr_{si}_{J}")
                    nc.gpsimd.partition_all_reduce(
                        srr[:], sq[:], channels=128, reduce_op=red_add)
                    lnt = scp.tile([128, 512], f32, tag="sln", bufs=2,
                                   name=f"lnt_{si}_{J}")
                    nc.scalar.activation(lnt[:], srr[:], AF.Ln,
                                         scale=1.0 / 128.0, bias=eps128[:])
                    rnb = scp.tile([128, 512], bf16, tag="rnb", bufs=4,
                                   name=f"rnb_{si}_{J}")
                    nc.scalar.activation(rnb[:], lnt[:], AF.Exp, scale=-0.5)
                    rnbs.append((J, rnb))
                    if dbg and pair == 0:
                        nc.sync.dma_start(
                            dbg["rnb"][:, (wi * 2 + hl) * T + J * 512:
                                       (wi * 2 + hl) * T + (J + 1) * 512],
                            rnb[:])
                flush(pend_norm)

                def norm(dest=dest, hl=hl, rnbs=rnbs):
                    for J, rnb in rnbs:
                        dcol = hl * T + J * 512
                        nc.vector.tensor_tensor(
                            dest[:, dcol:dcol + 512],
                            dest[:, dcol:dcol + 512],
                            rnb[:], op=ALU.mult)
                pend_norm[0] = norm
        flush(pend_norm)
        if dbg and pair == 0:
            nc.sync.dma_start(dbg["q"][:], q_sb[:])
            nc.sync.dma_start(dbg["k"][:], k_sb[:])

        # --- 2b: v for both heads; each accumulation chain owns a full PSUM
        # bank (matmul start=True zeroes the whole bank, so chains must not
        # share one): quarters 0 and 2 of two [128,1024] tiles = 4 banks. ---
        for tq in range(4):
            vps = []
            for q4 in range(2):
                vps.append(psp.tile([128, 1024], f32, tag="s", bufs=2,
                                    name=f"vps_{pair}_{tq}_{q4}"))
            for i in range(NCT):
                for t in range(4):
                    tt = tq * 4 + t
                    nc.tensor.matmul(
                        vps[t // 2][:, (t % 2) * 512:(t % 2) * 512 + 256],
                        xts[:, i * T + tt * 128:i * T + (tt + 1) * 128],
                        wvp[:, i * 256:(i + 1) * 256],
                        start=(i == 0), stop=(i == NCT - 1))
            for t in range(4):
                tt = tq * 4 + t
                nc.vector.tensor_copy(
                    v_sb[:, tt * 256:(tt + 1) * 256],
                    vps[t // 2][:, (t % 2) * 512:(t % 2) * 512 + 256])

        if dbg and pair == 0:
            nc.sync.dma_start(dbg["v"][:], v_sb[:])

        # --- 2c: attention, software-pipelined ---
        for hl in range(2):
            h = pair * 2 + hl
            for J in range(NT):
                nI = 4 * J + 4
                yps = psp.tile([128, 512], f32, tag="y", bufs=2,
                               name=f"yps_{pair}_{hl}_{J}")
                dps = psp.tile([1, 512], f32, tag="d", bufs=2,
                               name=f"dps_{pair}_{hl}_{J}")
                prev = [None]

                def consume(ex2, I0, yps=yps, dps=dps, hl=hl, nI=nI):
                    for half2 in range(2):
                        I = I0 + half2
                        exsl = ex2[:, half2 * 512:(half2 + 1) * 512]
                        nc.tensor.matmul(
                            yps[:],
                            v_sb[:, I * 256 + hl * 128:I * 256 + hl * 128 + 128],
                            exsl, start=(I == 0), stop=(I == nI - 1))
                        nc.tensor.matmul(
                            dps[:], onesA[:], exsl,
                            start=(I == 0), stop=(I == nI - 1))

                for Ip in range(nI // 2):
                    sps2 = psp.tile([128, 1024], f32, tag="s", bufs=2,
                                    name=f"sps_{pair}_{hl}_{J}_{Ip}")
                    for half2 in range(2):
                        I = 2 * Ip + half2
                        diag = I >= 4 * J
                        osl = sps2[:, half2 * 512:(half2 + 1) * 512]
                        nc.tensor.matmul(
                            osl,
                            k_sb[:, hl * T + I * 128:hl * T + (I + 1) * 128],
                            q_sb[:, hl * T + J * 512:hl * T + (J + 1) * 512],
                            start=True, stop=(not diag))
                        if diag:
                            r = I - 4 * J
                            nc.tensor.matmul(
                                osl, trilA[:], maskB[:, r * 512:(r + 1) * 512],
                                start=False, stop=True)
                    ex2 = scp.tile([128, 1024], bf16, tag="ex", bufs=3,
                                   name=f"ex_{pair}_{hl}_{J}_{Ip}")
                    nc.scalar.activation(ex2[:], sps2[:], AF.Exp, scale=SCL)
                    if prev[0] is not None:
                        consume(*prev[0])
                    if Ip == 0:
                        flush(pend_epi)
                    prev[0] = (ex2, 2 * Ip)
                consume(*prev[0])

                def epilogue(yps=yps, dps=dps, h=h, J=J):
                    rowt = rowp.tile([1, 512], f32, tag="rc", bufs=2,
                                     name=f"rc_{h}_{J}")
                    nc.vector.reciprocal_approx_fast(out=rowt[:], in_=dps[:])
                    rb = scp.tile([128, 512], f32, tag="rbc", bufs=2,
                                  name=f"rb_{h}_{J}")
                    nc.gpsimd.partition_broadcast(rb[:], rowt[:])
                    yt = scp.tile([128, 512], bf16, tag="yt", bufs=2,
                                  name=f"yt_{h}_{J}")
                    nc.vector.tensor_tensor(yt[:], yps[:], rb[:], op=ALU.mult)
                    nc.sync.dma_start(
                        y_d[:, h * T + J * 512:h * T + (J + 1) * 512], yt[:])
                    if dbg:
                        nc.sync.dma_start(
                            dbg["y"][:, h * T + J * 512:h * T + (J + 1) * 512],
                            yt[:])
                        if h < 2:
                            nc.sync.dma_start(
                                dbg["den"][:, (h * 4 + J) * 512:
                                           (h * 4 + J + 1) * 512],
                                rowt[:])
                pend_epi[0] = epilogue
        flush(pend_epi)


def _host_prep(inputs):
    bf = ml_dtypes.bfloat16
    x = np.asarray(inputs["x"], dtype=np.float32)
    Wq = np.asarray(inputs["Wq"], dtype=np.float32)
    Wk = np.asarray(inputs["Wk"], dtype=np.float32)
    Wv = np.asarray(inputs["Wv"], dtype=np.float32)
    Wo = np.asarray(inputs["Wo"], dtype=np.float32)
    w_omega = np.asarray(inputs["w_omega"], dtype=np.float32)
    b_omega = np.asarray(inputs["b_omega"], dtype=np.float32)
    log_freq = np.asarray(inputs["log_freq"], dtype=np.float32)
    q_gamma = np.asarray(inputs["q_gamma"], dtype=np.float32)
    k_gamma = np.asarray(inputs["k_gamma"], dtype=np.float32)

    womg = w_omega.reshape(NCT, 128).T.astype(bf)  # [p, i] = w_omega[i*128+p]
    b16 = (b_omega / 16.0).reshape(1, 1).astype(np.float32)
    logf2 = np.concatenate([log_freq, log_freq]).reshape(128, 1)
    gqv = q_gamma.reshape(128, 1).astype(np.float32)
    gqB = np.concatenate([q_gamma[:DH], -q_gamma[DH:]]).reshape(128, 1)
    gkv = k_gamma.reshape(128, 1).astype(np.float32)
    gkB = np.concatenate([k_gamma[:DH], -k_gamma[DH:]]).reshape(128, 1)
    kk = np.arange(128)
    trilA = (kk[:, None] <= kk[None, :]).astype(bf)  # [k, p] = (k <= p)
    p = np.arange(128)[:, None]
    c = np.arange(512)[None, :]
    maskB = np.concatenate(
        [(NEG * ((p + r * 128) > c)).astype(np.float32) for r in range(4)],
        axis=1).astype(bf)
    onesA = np.ones((128, 1), dtype=bf)

    in_maps = []
    for core in range(8):
        b, g = core // 2, core % 2
        in_maps.append({
            "xt": np.ascontiguousarray(x[b].T).astype(bf),
            "wq": np.ascontiguousarray(Wq[g * GD:(g + 1) * GD, :].T).astype(bf),
            "wk": np.ascontiguousarray(Wk[g * GD:(g + 1) * GD, :].T).astype(bf),
            "wv": np.ascontiguousarray(Wv[g * GD:(g + 1) * GD, :].T).astype(bf),
            "wo": np.ascontiguousarray(Wo[:, g * GD:(g + 1) * GD].T).astype(bf),
            "womg": womg, "b16": b16,
            "logf2": logf2.astype(np.float32),
            "gq": gqv, "gqB": gqB.astype(np.float32),
            "gk": gkv, "gkB": gkB.astype(np.float32),
            "trilA": trilA, "maskB": maskB, "onesA": onesA,
        })
    return in_maps


def kernel(**inputs) -> np.ndarray:
    if "nc" not in _CACHE:
        _CACHE["nc"] = _build()
    nc = _CACHE["nc"]
    in_maps = _host_prep(inputs)
    res = run_bass_kernel_spmd(nc, in_maps, core_ids=list(range(8)))
    out = np.empty((B, T, C), dtype=np.float32)
    for b in range(B):
        out[b] = res.results[2 * b]["out"] + res.results[2 * b + 1]["out"]
    return out


# revision 34
# speedup vs baseline: 1.6142x; 1.2244x over previous
"""Trainium2 Bass kernel for causal self-attention with cumulative-phase rotary
embedding (nn_CausalSelfAttention_64338610094602).

Sharding: 8 cores = 4 batches x 2 head-groups (tensor-parallel over heads).
Each core computes, for its (batch, 8-head group):
  omega/phi (replicated per batch), QKV projections, rotation + RMSNorm,
  causal attention (transposed-scores layout, max-free softmax), and a
  partial output projection. Host sums the two head-group partials per batch.

v2 design notes:
  - All big GEMM operands in bf16 (matmul rate keys off the moving operand;
    bf16 is 1 cycle/row at any N). PSUM accumulation stays fp32.
  - Rotation: gamma folded into per-J trig tiles; 4 DVE ops per site-J
    (A = qps*trigA, swapped-half B twice, one add writes q_sb/k_sb directly
    in bf16). No separate ACT eviction.
  - RMSNorm: Square on ACT, partition_all_reduce on GpSimd (PE no longer
    computes sum-of-squares); batched Ln/Exp per site on the replicated
    [128, 2048] reduce output produces the broadcast rstd directly.
  - Causal mask folded into the PE accumulation: for diagonal score tiles,
    one extra matmul (tril stationary x -1e9 step moving) adds -1e9*count
    to masked entries, so exp() gives exact zeros. No DVE mask multiply.
  - 2c softmax: scores for two key tiles accumulate into one [128,1024]
    PSUM tile (2 banks); a single wide ACT Exp halves ACT per-op overhead.
    Software-pipelined: next pair's scores are issued before this pair's
    yps/dps consumers so the in-order PE queue never waits on the Exp.
  - Softmax denominator reciprocal broadcast via GpSimd partition_broadcast
    (no K=1 PE matmuls); y stays resident in SBUF in bf16 (no DRAM spill).
  - PSUM budget: tagS 2x[128,1024] (4 banks) shared by projections/v/scores,
    tagY 2x[128,512] (yps + P3), tagD 2x[1,512] (omega + denominators).
"""
import math

import numpy as np
import ml_dtypes

import concourse.mybir as mybir
import concourse.tile as tile
from concourse import bacc
from concourse.bass_utils import run_bass_kernel_spmd

B, T, C = 4, 2048, 2048
H, D, DH = 16, 128, 64
HG = 8          # heads per core (head-group)
GD = HG * D     # group output dims = 1024
NT = T // 512   # 4 query blocks of 512
NCT = C // 128  # 16 contraction tiles
EPS = 1e-5
SCL = 1.0 / math.sqrt(D)
NEG = -1.0e9

dt = mybir.dt
AF = mybir.ActivationFunctionType
ALU = mybir.AluOpType

TWO_PI = 6.283185307179586
INV_2PI = 1.0 / TWO_PI
CW1 = float(np.float32(6.28125))
CW2 = float(np.float32(TWO_PI - 6.28125))
CW3 = float(TWO_PI - CW1 - float(np.float32(TWO_PI - 6.28125)))
MAGIC = 12582912.0  # 1.5 * 2^23: fp32 add/sub rounds to nearest int
HALF_PI = 1.5707963267948966
PI = 3.141592653589793

_CACHE = {}
DEBUG = False


def _build():
    from concourse import bass_isa
    red_add = bass_isa.ReduceOp.add

    f32, bf16 = dt.float32, dt.bfloat16
    nc = bacc.Bacc(None, target_bir_lowering=False)
    with tile.TileContext(nc) as tc:
        xt_d = nc.dram_tensor("xt", (C, T), bf16, kind="ExternalInput")
        wq_d = nc.dram_tensor("wq", (C, GD), bf16, kind="ExternalInput")
        wk_d = nc.dram_tensor("wk", (C, GD), bf16, kind="ExternalInput")
        wv_d = nc.dram_tensor("wv", (C, GD), bf16, kind="ExternalInput")
        wo_d = nc.dram_tensor("wo", (GD, C), bf16, kind="ExternalInput")
        womg_d = nc.dram_tensor("womg", (128, NCT), bf16, kind="ExternalInput")
        b16_d = nc.dram_tensor("b16", (1, 1), f32, kind="ExternalInput")
        logf2_d = nc.dram_tensor("logf2", (128, 1), f32, kind="ExternalInput")
        gq_d = nc.dram_tensor("gq", (128, 1), f32, kind="ExternalInput")
        gqB_d = nc.dram_tensor("gqB", (128, 1), f32, kind="ExternalInput")
        gk_d = nc.dram_tensor("gk", (128, 1), f32, kind="ExternalInput")
        gkB_d = nc.dram_tensor("gkB", (128, 1), f32, kind="ExternalInput")
        trilA_d = nc.dram_tensor("trilA", (128, 128), bf16, kind="ExternalInput")
        maskB_d = nc.dram_tensor("maskB", (128, 4 * 512), bf16, kind="ExternalInput")
        onesA_d = nc.dram_tensor("onesA", (128, 1), bf16, kind="ExternalInput")
        out_d = nc.dram_tensor("out", (T, C), f32, kind="ExternalOutput")
        dbg = {}
        if DEBUG:
            dbg["trig"] = nc.dram_tensor("dbg_trig", (128, 4 * T), bf16,
                                         kind="ExternalOutput")
            dbg["q"] = nc.dram_tensor("dbg_q", (128, 2 * T), bf16,
                                      kind="ExternalOutput")
            dbg["k"] = nc.dram_tensor("dbg_k", (128, 2 * T), bf16,
                                      kind="ExternalOutput")
            dbg["v"] = nc.dram_tensor("dbg_v", (128, 16 * 256), bf16,
                                      kind="ExternalOutput")
            dbg["y"] = nc.dram_tensor("dbg_y", (128, HG * T), bf16,
                                      kind="ExternalOutput")
            dbg["den"] = nc.dram_tensor("dbg_den", (1, 8 * 512), f32,
                                        kind="ExternalOutput")
            dbg["rnb"] = nc.dram_tensor("dbg_rnb", (128, 4 * T), bf16,
                                        kind="ExternalOutput")

        with tc.tile_pool(name="const", bufs=1) as constp, \
             tc.tile_pool(name="dram", bufs=1, space="DRAM") as dramp, \
             tc.tile_pool(name="psp", bufs=1, space="PSUM") as psp:

            # ---- constants ----
            womg = constp.tile([128, NCT], bf16)
            nc.sync.dma_start(womg[:], womg_d[:])
            b16t = constp.tile([1, 1], f32)
            nc.sync.dma_start(b16t[:], b16_d[:])
            logf2 = constp.tile([128, 1], f32)
            nc.sync.dma_start(logf2[:], logf2_d[:])
            gq = constp.tile([128, 1], f32)
            nc.sync.dma_start(gq[:], gq_d[:])
            gqB = constp.tile([128, 1], f32)
            nc.sync.dma_start(gqB[:], gqB_d[:])
            gk = constp.tile([128, 1], f32)
            nc.sync.dma_start(gk[:], gk_d[:])
            gkB = constp.tile([128, 1], f32)
            nc.sync.dma_start(gkB[:], gkB_d[:])
            trilA = constp.tile([128, 128], bf16)
            nc.sync.dma_start(trilA[:], trilA_d[:])
            maskB = constp.tile([128, 4 * 512], bf16)
            nc.sync.dma_start(maskB[:], maskB_d[:])
            onesA = constp.tile([128, 1], bf16)
            nc.sync.dma_start(onesA[:], onesA_d[:])
            freq2 = constp.tile([128, 1], f32)
            nc.scalar.activation(freq2[:], logf2[:], AF.Exp)
            eps128 = constp.tile([128, 1], f32)
            nc.vector.memset(eps128[:], EPS)

            y_d = dramp.tile([128, HG * T], bf16)  # yT per head at col h*T

            with tc.tile_pool(name="big", bufs=1) as bigp, \
                 tc.tile_pool(name="xtp", bufs=1) as xtp, \
                 tc.tile_pool(name="wstp", bufs=1) as wstp, \
                 tc.tile_pool(name="scp", bufs=1) as scp, \
                 tc.tile_pool(name="rowp", bufs=1) as rowp:
                # pair-persistent SBUF state
                q_sb = bigp.tile([128, 2 * T], bf16)   # [D, hl*T + t]
                k_sb = bigp.tile([128, 2 * T], bf16)
                v_sb = bigp.tile([128, 16 * 256], bf16)  # key tile tt at tt*256
                trigA_q = bigp.tile([128, T], bf16)
                trigB_q = bigp.tile([128, T], bf16)
                trigA_k = bigp.tile([128, T], bf16)
                trigB_k = bigp.tile([128, T], bf16)
                _main(nc, tc, xt_d, wq_d, wk_d, wv_d, red_add,
                      xtp, wstp, scp, rowp, psp,
                      womg, b16t, freq2, gq, gqB, gk, gkB, trilA, maskB,
                      onesA, eps128, q_sb, k_sb, v_sb, y_d,
                      trigA_q, trigB_q, trigA_k, trigB_k, dbg)

            # ---- P3: output projection out = y^T W_o (partial over heads) ----
            with tc.tile_pool(name="p3w", bufs=1) as p3w, \
                 tc.tile_pool(name="p3o", bufs=1) as p3o:
                wosb = p3w.tile([128, HG * C], bf16, name="wosb")
                for cb in range(C // 512):
                    for hh in range(HG):
                        nc.sync.dma_start(
                            wosb[:, hh * C + cb * 512:hh * C + (cb + 1) * 512],
                            wo_d[hh * 128:(hh + 1) * 128,
                                 cb * 512:(cb + 1) * 512])
                y_slots = [None, None]

                def issue_y(ti):
                    yti = p3w.tile([128, HG * 128], bf16, tag="yt", bufs=2,
                                   name=f"yti_{ti}")
                    for hh in range(HG):
                        nc.sync.dma_start(
                            yti[:, hh * 128:(hh + 1) * 128],
                            y_d[:, hh * T + ti * 128:hh * T + (ti + 1) * 128])
                    y_slots[ti % 2] = yti

                issue_y(0)
                for ti in range(T // 128):
                    if ti + 1 < T // 128:
                        issue_y(ti + 1)
                    yti = y_slots[ti % 2]
                    for cb in range(C // 512):
                        ops = psp.tile([128, 512], f32, tag="y", bufs=2,
                                       name=f"ops_{cb}_{ti}")
                        for hh in range(HG):
                            nc.tensor.matmul(
                                ops[:],
                                yti[:, hh * 128:(hh + 1) * 128],
                                wosb[:, hh * C + cb * 512:hh * C + (cb + 1) * 512],
                                start=(hh == 0), stop=(hh == HG - 1))
                        osb = p3o.tile([128, 512], f32, tag="osb", bufs=3)
                        nc.scalar.copy(osb[:], ops[:])
                        nc.sync.dma_start(
                            out_d[ti * 128:(ti + 1) * 128,
                                  cb * 512:(cb + 1) * 512],
                            osb[:])
    nc.compile()
    return nc


def _main(nc, tc, xt_d, wq_d, wk_d, wv_d, red_add,
          xtp, wstp, scp, rowp, psp,
          womg, b16t, freq2, gq, gqB, gk, gkB, trilA, maskB,
          onesA, eps128, q_sb, k_sb, v_sb, y_d,
          trigA_q, trigB_q, trigA_k, trigB_k, dbg):
    f32, bf16 = dt.float32, dt.bfloat16

    xts = xtp.tile([128, NCT * T], bf16)  # c-tile i at cols [i*T,(i+1)*T)
    for i in range(NCT):
        nc.sync.dma_start(xts[:, i * T:(i + 1) * T],
                          xt_d[i * 128:(i + 1) * 128, :])

    # ---- P1: omega -> phi -> trig (PE does only the omega matvecs) ----
    with tc.tile_pool(name="p1p", bufs=1) as p1p:
        omega = rowp.tile([1, T], f32, tag="om")
        for J in range(NT):
            omps = psp.tile([1, 512], f32, tag="d", bufs=2, name=f"omps_{J}")
            for i in range(NCT):
                nc.tensor.matmul(
                    omps[:], womg[:, i:i + 1],
                    xts[:, i * T + J * 512:i * T + J * 512 + 512],
                    start=(i == 0), stop=(i == NCT - 1))
            nc.scalar.activation(omega[:, J * 512:(J + 1) * 512], omps[:],
                                 AF.Sigmoid, scale=1.0 / 16.0, bias=b16t[:])
        incl = rowp.tile([1, T], f32, tag="incl")
        nc.vector.tensor_tensor_scan(incl[:], omega[:], omega[:], 0.0,
                                     ALU.add, ALU.bypass)
        phi = rowp.tile([1, T], f32, tag="phi")
        nc.vector.tensor_sub(phi[:], incl[:], omega[:])
        for J in range(NT):
            sl = slice(J * 512, (J + 1) * 512)
            phi2 = p1p.tile([128, 512], f32, tag="p1", bufs=4,
                            name=f"phi2_{J}")
            nc.gpsimd.partition_broadcast(phi2[:], phi[:, sl])
            ang = p1p.tile([128, 512], f32, tag="p1", bufs=4, name=f"ang_{J}")
            nc.vector.tensor_scalar(ang[:], phi2[:], freq2[:], None,
                                    op0=ALU.mult)
            mm = p1p.tile([128, 512], f32, tag="p1", bufs=4, name=f"mm_{J}")
            nc.vector.tensor_scalar(mm[:], ang[:], INV_2PI, MAGIC,
                                    op0=ALU.mult, op1=ALU.add)
            kk = p1p.tile([128, 512], f32, tag="p1", bufs=4, name=f"kk_{J}")
            nc.vector.tensor_scalar_add(kk[:], mm[:], -MAGIC)
            red = p1p.tile([128, 512], f32, tag="p1", bufs=4, name=f"red_{J}")
            nc.vector.cody_waite_cascade(red[:], ang[:], kk[:], CW1, CW2, CW3)
            red2 = p1p.tile([128, 512], f32, tag="p1", bufs=4,
                            name=f"red2_{J}")
            nc.vector.add_range_wrap(red2[:], red[:], HALF_PI, PI, TWO_PI)
            sinr = p1p.tile([128, 512], f32, tag="p1", bufs=4,
                            name=f"sinr_{J}")
            nc.scalar.activation(sinr[:], red[:], AF.Sin)
            cosr = p1p.tile([128, 512], f32, tag="p1", bufs=4,
                            name=f"cosr_{J}")
            nc.scalar.activation(cosr[:], red2[:], AF.Sin)
            # gamma-folded trig tiles (bf16)
            nc.scalar.activation(trigA_q[:, sl], cosr[:], AF.Copy, scale=gq[:])
            nc.scalar.activation(trigB_q[:, sl], sinr[:], AF.Copy, scale=gqB[:])
            nc.scalar.activation(trigA_k[:, sl], cosr[:], AF.Copy, scale=gk[:])
            nc.scalar.activation(trigB_k[:, sl], sinr[:], AF.Copy, scale=gkB[:])
        if dbg:
            for ii, tt_ in enumerate((trigA_q, trigB_q, trigA_k, trigB_k)):
                nc.sync.dma_start(dbg["trig"][:, ii * T:(ii + 1) * T], tt_[:])

    # ---- P2 per pair ----
    pend_norm = [None]
    pend_epi = [None]
    pend_ssq = [None]

    def flush(pend):
        if pend[0] is not None:
            pend[0]()
            pend[0] = None

    sites = [(pair, wi, hl) for pair in range(4) for wi in range(2)
             for hl in range(2)]
    wp_slots = [None, None]

    def issue_panel(si):
        pair, wi, hl = sites[si]
        h = pair * 2 + hl
        w_d = (wq_d, wk_d)[wi]
        wp = wstp.tile([128, NCT * 128], bf16, tag="wp", bufs=2,
                       name=f"wp_{si}")
        for i in range(NCT):
            nc.sync.dma_start(
                wp[:, i * 128:(i + 1) * 128],
                w_d[i * 128:(i + 1) * 128, h * 128:(h + 1) * 128])
        wp_slots[si % 2] = wp

    issue_panel(0)

    for pair in range(4):
        # wv panel for this pair (resident; streamed during 2a)
        wvp = wstp.tile([128, NCT * 256], bf16, tag="wvp", bufs=1,
                        name=f"wvp_{pair}")
        for i in range(NCT):
            nc.sync.dma_start(
                wvp[:, i * 256:(i + 1) * 256],
                wv_d[i * 128:(i + 1) * 128, pair * 256:(pair + 1) * 256])

        # --- 2a: q/k for both heads ---
        for wi in range(2):
            for hl in range(2):
                si = pair * 4 + wi * 2 + hl
                if si + 1 < len(sites):
                    issue_panel(si + 1)
                wp = wp_slots[si % 2]
                trigA = (trigA_q, trigA_k)[wi]
                trigB = (trigB_q, trigB_k)[wi]
                dest = (q_sb, k_sb)[wi]
                flush(pend_ssq)
                sqs = []
                for Jp in range(2):
                    qps2 = psp.tile([128, 1024], f32, tag="s", bufs=2,
                                    name=f"qps2_{si}_{Jp}")
                    for i in range(NCT):
                        for Jh in range(2):
                            J = 2 * Jp + Jh
                            nc.tensor.matmul(
                                qps2[:, Jh * 512:(Jh + 1) * 512],
                                wp[:, i * 128:(i + 1) * 128],
                                xts[:, i * T + J * 512:i * T + J * 512 + 512],
                                start=(i == 0), stop=(i == NCT - 1))
                    for Jh in range(2):
                        J = 2 * Jp + Jh
                        qsl = qps2[:, Jh * 512:(Jh + 1) * 512]
                        sl = slice(J * 512, (J + 1) * 512)
                        dcol = hl * T + J * 512
                        # rotation: A + swapped-half B, gamma folded in trig
                        A = scp.tile([128, 512], f32, tag="ra", bufs=2,
                                     name=f"A_{si}_{J}")
                        nc.vector.tensor_tensor(A[:], qsl, trigA[:, sl],
                                                op=ALU.mult)
                        Bt = scp.tile([128, 512], f32, tag="rb", bufs=2,
                                      name=f"Bt_{si}_{J}")
                        nc.vector.tensor_tensor(
                            Bt[0:DH, :],
                            qps2[DH:128, Jh * 512:(Jh + 1) * 512],
                            trigB[0:DH, sl], op=ALU.mult)
                        nc.vector.tensor_tensor(
                            Bt[DH:128, :],
                            qps2[0:DH, Jh * 512:(Jh + 1) * 512],
                            trigB[DH:128, sl], op=ALU.mult)
                        nc.vector.tensor_add(
                            dest[:, dcol:dcol + 512], A[:], Bt[:])
                        # sum-of-squares path (rotation preserves norms)
                        sq = scp.tile([128, 512], bf16, tag="sq", bufs=4,
                                      name=f"sq_{si}_{J}")
                        nc.scalar.activation(sq[:], qsl, AF.Square)
                        sqs.append((J, sq))
                # deferred rstd: PE colsum of sq (M=1 matmul), one-op
                # Abs_reciprocal_sqrt, GpSimd broadcast — emitted at the NEXT
                # site so the PE never waits on the ACT Square chain.
                rnbs = []

                def ssq_tail(sqs=sqs, rnbs=rnbs, si=si, wi=wi, hl=hl):
                    for J, sq in sqs:
                        ssqps = psp.tile([1, 512], f32, tag="d", bufs=2,
                                         name=f"ssq_{si}_{J}")
                        nc.tensor.matmul(ssqps[:], onesA[:], sq[:],
                                         start=True, stop=True)
                        rrow = rowp.tile([1, 512], bf16, tag="rr", bufs=2,
                                         name=f"rrow_{si}_{J}")
                        nc.scalar.activation(rrow[:], ssqps[:],
                                             AF.Abs_reciprocal_sqrt,
                                             scale=1.0 / 128.0,
                                             bias=eps128[0:1, :])
                        rnb = scp.tile([128, 512], bf16, tag="rnb", bufs=4,
                                       name=f"rnb_{si}_{J}")
                        nc.gpsimd.partition_broadcast(rnb[:], rrow[:])
                        rnbs.append((J, rnb))
                        if dbg and pair == 0:
                            nc.sync.dma_start(
                                dbg["rnb"][:, (wi * 2 + hl) * T + J * 512:
                                           (wi * 2 + hl) * T + (J + 1) * 512],
                                rnb[:])
                pend_ssq[0] = ssq_tail
                flush(pend_norm)

                def norm(dest=dest, hl=hl, rnbs=rnbs):
                    for J, rnb in rnbs:
                        dcol = hl * T + J * 512
                        nc.vector.tensor_tensor(
                            dest[:, dcol:dcol + 512],
                            dest[:, dcol:dcol + 512],
                            rnb[:], op=ALU.mult)
                pend_norm[0] = norm
        flush(pend_ssq)
        flush(pend_norm)
        if dbg and pair == 0:
            nc.sync.dma_start(dbg["q"][:], q_sb[:])
            nc.sync.dma_start(dbg["k"][:], k_sb[:])

        # --- 2b: v for both heads; each accumulation chain owns a full PSUM
        # bank (matmul start=True zeroes the whole bank, so chains must not
        # share one): quarters 0 and 2 of two [128,1024] tiles = 4 banks. ---
        for tq in range(4):
            vps = []
            for q4 in range(2):
                vps.append(psp.tile([128, 1024], f32, tag="s", bufs=2,
                                    name=f"vps_{pair}_{tq}_{q4}"))
            for i in range(NCT):
                for t in range(4):
                    tt = tq * 4 + t
                    nc.tensor.matmul(
                        vps[t // 2][:, (t % 2) * 512:(t % 2) * 512 + 256],
                        xts[:, i * T + tt * 128:i * T + (tt + 1) * 128],
                        wvp[:, i * 256:(i + 1) * 256],
                        start=(i == 0), stop=(i == NCT - 1))
            for t in range(4):
                tt = tq * 4 + t
                nc.vector.tensor_copy(
                    v_sb[:, tt * 256:(tt + 1) * 256],
                    vps[t // 2][:, (t % 2) * 512:(t % 2) * 512 + 256])

        if dbg and pair == 0:
            nc.sync.dma_start(dbg["v"][:], v_sb[:])

        # --- 2c: attention, software-pipelined ---
        for hl in range(2):
            h = pair * 2 + hl
            for J in range(NT):
                nI = 4 * J + 4
                yps = psp.tile([128, 512], f32, tag="y", bufs=2,
                               name=f"yps_{pair}_{hl}_{J}")
                dps = psp.tile([1, 512], f32, tag="d", bufs=2,
                               name=f"dps_{pair}_{hl}_{J}")
                prev = [None]

                def consume(ex2, I0, yps=yps, dps=dps, hl=hl, nI=nI):
                    for half2 in range(2):
                        I = I0 + half2
                        exsl = ex2[:, half2 * 512:(half2 + 1) * 512]
                        nc.tensor.matmul(
                            yps[:],
                            v_sb[:, I * 256 + hl * 128:I * 256 + hl * 128 + 128],
                            exsl, start=(I == 0), stop=(I == nI - 1))
                        nc.tensor.matmul(
                            dps[:], onesA[:], exsl,
                            start=(I == 0), stop=(I == nI - 1))

                for Ip in range(nI // 2):
                    sps2 = psp.tile([128, 1024], f32, tag="s", bufs=2,
                                    name=f"sps_{pair}_{hl}_{J}_{Ip}")
                    for half2 in range(2):
                        I = 2 * Ip + half2
                        diag = I >= 4 * J
                        osl = sps2[:, half2 * 512:(half2 + 1) * 512]
                        nc.tensor.matmul(
                            osl,
                            k_sb[:, hl * T + I * 128:hl * T + (I + 1) * 128],
                            q_sb[:, hl * T + J * 512:hl * T + (J + 1) * 512],
                            start=True, stop=(not diag))
                        if diag:
                            r = I - 4 * J
                            nc.tensor.matmul(
                                osl, trilA[:], maskB[:, r * 512:(r + 1) * 512],
                                start=False, stop=True)
                    ex2 = scp.tile([128, 1024], bf16, tag="ex", bufs=3,
                                   name=f"ex_{pair}_{hl}_{J}_{Ip}")
                    nc.scalar.activation(ex2[:], sps2[:], AF.Exp, scale=SCL)
                    if prev[0] is not None:
                        consume(*prev[0])
                    if Ip == 0:
                        flush(pend_epi)
                    prev[0] = (ex2, 2 * Ip)
                consume(*prev[0])

                def epilogue(yps=yps, dps=dps, h=h, J=J):
                    rowt = rowp.tile([1, 512], f32, tag="rc", bufs=2,
                                     name=f"rc_{h}_{J}")
                    nc.vector.reciprocal_approx_fast(out=rowt[:], in_=dps[:])
                    rb = scp.tile([128, 512], f32, tag="rbc", bufs=2,
                                  name=f"rb_{h}_{J}")
                    nc.gpsimd.partition_broadcast(rb[:], rowt[:])
                    yt = scp.tile([128, 512], bf16, tag="yt", bufs=2,
                                  name=f"yt_{h}_{J}")
                    nc.vector.tensor_tensor(yt[:], yps[:], rb[:], op=ALU.mult)
                    nc.sync.dma_start(
                        y_d[:, h * T + J * 512:h * T + (J + 1) * 512], yt[:])
                    if dbg:
                        nc.sync.dma_start(
                            dbg["y"][:, h * T + J * 512:h * T + (J + 1) * 512],
                            yt[:])
                        if h < 2:
                            nc.sync.dma_start(
                                dbg["den"][:, (h * 4 + J) * 512:
                                           (h * 4 + J + 1) * 512],
                                rowt[:])
                pend_epi[0] = epilogue
        flush(pend_epi)


def _host_prep(inputs):
    bf = ml_dtypes.bfloat16
    x = np.asarray(inputs["x"], dtype=np.float32)
    Wq = np.asarray(inputs["Wq"], dtype=np.float32)
    Wk = np.asarray(inputs["Wk"], dtype=np.float32)
    Wv = np.asarray(inputs["Wv"], dtype=np.float32)
    Wo = np.asarray(inputs["Wo"], dtype=np.float32)
    w_omega = np.asarray(inputs["w_omega"], dtype=np.float32)
    b_omega = np.asarray(inputs["b_omega"], dtype=np.float32)
    log_freq = np.asarray(inputs["log_freq"], dtype=np.float32)
    q_gamma = np.asarray(inputs["q_gamma"], dtype=np.float32)
    k_gamma = np.asarray(inputs["k_gamma"], dtype=np.float32)

    womg = w_omega.reshape(NCT, 128).T.astype(bf)  # [p, i] = w_omega[i*128+p]
    b16 = (b_omega / 16.0).reshape(1, 1).astype(np.float32)
    logf2 = np.concatenate([log_freq, log_freq]).reshape(128, 1)
    gqv = q_gamma.reshape(128, 1).astype(np.float32)
    gqB = np.concatenate([q_gamma[:DH], -q_gamma[DH:]]).reshape(128, 1)
    gkv = k_gamma.reshape(128, 1).astype(np.float32)
    gkB = np.concatenate([k_gamma[:DH], -k_gamma[DH:]]).reshape(128, 1)
    kk = np.arange(128)
    trilA = (kk[:, None] <= kk[None, :]).astype(bf)  # [k, p] = (k <= p)
    p = np.arange(128)[:, None]
    c = np.arange(512)[None, :]
    maskB = np.concatenate(
        [(NEG * ((p + r * 128) > c)).astype(np.float32) for r in range(4)],
        axis=1).astype(bf)
    onesA = np.ones((128, 1), dtype=bf)

    in_maps = []
    for core in range(8):
        b, g = core // 2, core % 2
        in_maps.append({
            "xt": np.ascontiguousarray(x[b].T).astype(bf),
            "wq": np.ascontiguousarray(Wq[g * GD:(g + 1) * GD, :].T).astype(bf),
            "wk": np.ascontiguousarray(Wk[g * GD:(g + 1) * GD, :].T).astype(bf),
            "wv": np.ascontiguousarray(Wv[g * GD:(g + 1) * GD, :].T).astype(bf),
            "wo": np.ascontiguousarray(Wo[:, g * GD:(g + 1) * GD].T).astype(bf),
            "womg": womg, "b16": b16,
            "logf2": logf2.astype(np.float32),
            "gq": gqv, "gqB": gqB.astype(np.float32),
            "gk": gkv, "gkB": gkB.astype(np.float32),
            "trilA": trilA, "maskB": maskB, "onesA": onesA,
        })
    return in_maps


def kernel(**inputs) -> np.ndarray:
    if "nc" not in _CACHE:
        _CACHE["nc"] = _build()
    nc = _CACHE["nc"]
    in_maps = _host_prep(inputs)
    res = run_bass_kernel_spmd(nc, in_maps, core_ids=list(range(8)))
    out = np.empty((B, T, C), dtype=np.float32)
    for b in range(B):
        out[b] = res.results[2 * b]["out"] + res.results[2 * b + 1]["out"]
    return out
